# revision 15
# baseline (speedup 1.0000x reference)
"""Single-head attention (B=8, T=4096, E=768, H=64) on 8 TRN2 NeuronCores.

Sharding: data-parallel over batch B - one batch element per core, Q/K/V
projection weights replicated. Per core:

  phase 1: SWDGE cast-loads x as bf16; PE-transpose 128x128 blocks -> xT
           [E,T] in SBUF (bf16 rate, 4 blocks batched per PSUM tile so DVE
           drains them in one copy)
  phase 2: packed projections: stationary [Wq|Wk] -> one matmul emits q on
           psum partitions 0:64 and k on 64:128; ACT drains q -> qT(lo) and
           k -> kT(hi). v separately -> vT (fp8e4 when KERNEL_FP8MM2).
           Per-chunk SBUF->SBUF DMAs then mirror qT to partitions 64:128
           and kT to 0:64 so MM1 can run 2-way row-packed (PE array rows
           0:63 and 64:127 concurrently).
  phase 3: PE-transpose vT -> v tiles [128s, 64], append ones column
           (row 64 of MM2 output accumulates the softmax denominator).
           v1 is laid out [pair, 2, 65] so one fp8 DoubleRow matmul covers
           an s-tile pair.
  phase 4 (software-pipelined over 16 s-tile PAIRS x 8 q-groups):
           MM1 pair: S^T halves for s0 (rows 0:63) and s1 (rows 64:127)
             land in one contiguous [128, 2, 512] 2-bank PSUM tile
           exp of the whole 1024-wide tile runs as ONE engine call,
             alternating pairs between ACT (hardware Exp, free scale) and
             DVE (custom fused monic-cubic + 3 squarings = exp(8w)); the
             1024-wide calls amortize the per-instruction overheads
             (ACT 352cyc, DVE 120cyc) that dominated 512-wide halves.
           MM2: one fp8e4 DoubleRow matmul per pair:
             out^T [65, 512t] += sum_i v1[:,p,i,:].T @ ex[:,i,:]
           tail per group: PE-transpose out^T blocks, multiply by
             reciprocal of the denominator row, DMA [t, h] to DRAM.

  Matmul-facing q/k are bf16; v path and exp output are fp8e4 (att weights
  in [e^-2, e^2.2], v ~N(0,0.58) - both comfortably inside e4m3 range; the
  softmax numerator/denominator ratio cancels most quantization noise).
"""

import math
import os
import sys

for _p in ("/opt/trn_rl_repo", "/root/.axon_site/_ro/trn_rl_repo"):
    if os.path.isdir(_p) and _p not in sys.path:
        sys.path.insert(0, _p)

import numpy as np

import concourse.bacc as bacc
import concourse.tile as tile
from concourse import mybir
from concourse.bass_utils import run_bass_kernel_spmd
from concourse.masks import make_identity

B, T, E, H = 8, 4096, 768, 64
P = 128
NE = E // P            # 6 e-chunks
NT = T // P            # 32 s-tiles
GQ = 512               # q-group width (t rows)
NG = T // GQ           # 8 q-groups
NPAIR = NT // 2        # 16 s-tile pairs per q-group
SCALE = float(H) ** -0.5

F32 = mybir.dt.float32
BF16 = mybir.dt.bfloat16
FP8 = mybir.dt.float8e4

# --- custom DVE exp: q(v) = (v+A)((v+B)^2 + C), out = q^8 ------------------
# Fit of e^w deg-3 (relative-error weighted) on |w| <= 0.36, monic form via
# v = LAM*w with LAM = cbrt(c3). stp holds v = logits*LAM/8 (folded into the
# qT prescale); ACT path recovers exp(logits) via its free scale 8/LAM.
EXP_A = 0.890217935821643
EXP_B = 0.3930562704875204
EXP_C = 0.9687638651114505
LAM = 0.5486231552172741

_EXP8_OP = None


def _register_exp8():
    global _EXP8_OP
    if _EXP8_OP is not None:
        return _EXP8_OP
    import concourse.dve_ops as dvo
    from concourse.dve_spec import C0, C1, C2, Spec, Src0, lower
    from concourse.dve_uop import DveOpSpec

    name = "EXP8_ATTN_ANT"
    for op in dvo.OPS:
        if op.name == name:
            _EXP8_OP = op
            return op

    t1 = Src0 + C0
    t2 = Src0 + C1
    t3 = t2 * t2
    t4 = t3 + C2
    q = t1 * t4
    r = q * q
    r2 = r * r
    body = r2 * r2

    def ref(in0, in1, s0, s1, imm2):
        x = in0.astype(np.float32)
        q = (x + s0) * ((x + s1) * (x + s1) + imm2)
        r = (q * q).astype(np.float32)
        r = (r * r).astype(np.float32)
        return (r * r).astype(np.float32)

    spec = Spec(body=body, reference=ref)
    row = dvo._CUSTOM_DVE_ROW_BASE + len(dvo.OPS)
    shas = {}
    for ver in ("v3", "v4"):
        s = DveOpSpec(name=name, opcode=row, uops=lower(spec, ver=ver),
                      rd1_en=False)
        shas[ver] = s.sha(ver)
    op = dvo.DveOp(name, spec, subdim=False, uops_sha=shas)
    dvo.OPS.append(op)
    dvo.CUSTOM_DVE_SPECS[name] = spec
    dvo._SUB_OPCODE_FOR_NAME[name] = row
    _EXP8_OP = op
    return op


def _dr32():
    """Pairs out of every 32 that run MM2 as fp8 DoubleRow (the rest run
    bf16). fp8 e4m3 att weights carry ~3.6% rms quantization noise that
    shows up as ~2.2e-2 absmax rel err at full fp8; the error scales as
    sqrt(f), so 16/32 keeps absmax ~1.6e-2 under the 2e-2 gate."""
    return max(0, min(32, int(os.environ.get("KERNEL_DR32", "16"))))


def _pair_fp8(idx, dr32):
    return ((idx + 1) * dr32) // 32 - (idx * dr32) // 32


def build_nc(reps=1, rep_scope="all"):
    nc = bacc.Bacc("TRN2", target_bir_lowering=False, debug=False, num_devices=8)

    x = nc.dram_tensor("x", [T, E], F32, kind="ExternalInput")
    wq = nc.dram_tensor("Wq", [E, H], F32, kind="ExternalInput")
    wk = nc.dram_tensor("Wk", [E, H], F32, kind="ExternalInput")
    wv = nc.dram_tensor("Wv", [E, H], F32, kind="ExternalInput")
    bq = nc.dram_tensor("bq", [H], F32, kind="ExternalInput")
    bk = nc.dram_tensor("bk", [H], F32, kind="ExternalInput")
    bv = nc.dram_tensor("bv", [H], F32, kind="ExternalInput")
    out = nc.dram_tensor("out", [T, H], F32, kind="ExternalOutput")

    dr32 = _dr32()

    with tile.TileContext(nc) as tc:
        with tc.tile_pool(name="consts", bufs=1) as consts:
            ident = consts.tile([P, P], F32)
            make_identity(nc, ident)
            identb = consts.tile([P, P], BF16, tag="identb")
            nc.vector.tensor_copy(identb, ident)
            identv = identb

            # packed [Wq | Wk] stationary (bf16) + [Wv | Wv] for col-tiled
            # cross-chunk pairing of the v projection
            wqk = consts.tile([P, NE, P], BF16, tag="wqk")
            wvt = consts.tile([P, NE, P], BF16, tag="wv")
            for cols, wdram, tag, wscale in (
                    (slice(0, H), wq, "fq", SCALE * LAM / 8.0),
                    (slice(H, P), wk, "fk", None)):
                wtf = consts.tile([P, NE, H], F32, tag="wf" + tag)
                nc.sync.dma_start(
                    out=wtf, in_=wdram[:, :].rearrange("(c p) h -> p c h", p=P)
                )
                if wscale is None:
                    nc.vector.tensor_copy(wqk[:, :, cols], wtf)
                else:
                    nc.scalar.mul(out=wqk[:, :, cols], in_=wtf, mul=wscale)
            wvf = consts.tile([P, NE, H], F32, tag="wfv")
            nc.sync.dma_start(
                out=wvf, in_=wv[:, :].rearrange("(c p) h -> p c h", p=P)
            )
            nc.vector.tensor_copy(wvt[:, :, 0:H], wvf)
            nc.vector.tensor_copy(wvt[:, :, H:P], wvf)

            # stacked bias: rows 0:64 = bq*(SCALE*LAM/8), rows 64:128 = bk
            bq_t = consts.tile([H, 1], F32, tag="bq")
            nc.sync.dma_start(out=bq_t, in_=bq[:].rearrange("(h o) -> h o", o=1))
            bqk = consts.tile([P, 1], F32, tag="bqk")
            nc.scalar.mul(out=bqk[0:H, :], in_=bq_t, mul=SCALE * LAM / 8.0)
            nc.sync.dma_start(
                out=bqk[H:P, :], in_=bk[:].rearrange("(h o) -> h o", o=1)
            )
            bv_t = consts.tile([P, 1], F32, tag="bv")
            nc.sync.dma_start(out=bv_t[0:H, :],
                              in_=bv[:].rearrange("(h o) -> h o", o=1))
            nc.sync.dma_start(out=bv_t[H:P, :],
                              in_=bv[:].rearrange("(h o) -> h o", o=1))

            with tc.tile_pool(name="persist", bufs=1) as persist:
                qTf = persist.tile([P, T], BF16, tag="qTf")
                kTf = persist.tile([P, T], BF16, tag="kTf")
                vT = persist.tile([P, T], BF16, tag="vT")
                # [pair, 2, W] so one matmul (pair for bf16) covers an
                # s-pair; fp8 planes are padded to 96 (32-col-strip LDW
                # rule): cols 0:63 = v, col 64 = ones, cols 65:95 = zeros
                v1 = v18 = None
                if dr32 < 32:
                    v1 = persist.tile([P, NPAIR, 2, H + 1], BF16, tag="v1")
                if dr32 > 0:
                    v18 = persist.tile([P, NPAIR, 2, VW8], FP8, tag="v18")

                if os.environ.get("KERNEL_FUSED", "0") == "1":
                    for _ in range(reps):
                        _fused(nc, tc, x, out, wqk, wvt, bqk, bv_t,
                               identb, identv, ident, qTf, kTf, vT, v1, v18)
                else:
                    setup_reps = reps if rep_scope in ("all", "setup") else 1
                    attn_reps = 1 if rep_scope == "setup" else reps
                    for _ in range(setup_reps):
                        _setup(nc, tc, x, wqk, wvt, bqk, bv_t, identb,
                               identv, qTf, kTf, vT, v1, v18)
                    for _ in range(attn_reps):
                        _attention(nc, tc, out, ident, qTf, kTf, v1, v18)
    nc.compile()
    return nc


def _emit_exp(nc, exp8, ex, stp, idx, dve32):
    """One whole-pair exp: stp [P,2,512] fp32 PSUM -> ex [P,2,512].
    Pairs alternate between DVE (custom exp8 cubic) and ACT (hardware Exp)
    via a Bresenham split: dve32 out of every 32 pairs go to DVE."""
    EXPF = mybir.ActivationFunctionType.Exp
    take_dve = ((idx + 1) * dve32) // 32 - (idx * dve32) // 32
    if take_dve:
        nc.vector._custom_dve(exp8, out=ex, in0=stp,
                              s0=EXP_A, s1=EXP_B, imm2=EXP_C)
    else:
        nc.scalar.activation(out=ex, in_=stp, func=EXPF, scale=8.0 / LAM)


VW8 = 96  # dual-fp8 LDW wants the plane width in 32-col strips


def _mm2(nc, outp, v1, v18, ex, p, pair_fp8):
    if pair_fp8:
        # dual-fp8 LDWEIGHTS requires the plane width to be a multiple of
        # 32 at tile_position (0,0), so the [v | ones] 65 columns are padded
        # to 96 (cols 65:95 are zeros; psum rows 65:95 accumulate zeros).
        # Row 64 still collects the softmax denominator. skip_group_check:
        # fp8 and bf16 pairs interleave in one accumulation group.
        nc.tensor.matmul(outp[0:VW8, :], v18[:, p, :, :], ex,
                         perf_mode=mybir.MatmulPerfMode.DoubleRow,
                         start=(p == 0), stop=(p == NPAIR - 1),
                         skip_group_check=True)
    else:
        nc.tensor.matmul(outp[0:H + 1, :], v1[:, p, 0, :], ex[:, 0, :],
                         start=(p == 0), stop=False, skip_group_check=True)
        nc.tensor.matmul(outp[0:H + 1, :], v1[:, p, 1, :], ex[:, 1, :],
                         start=False, stop=(p == NPAIR - 1),
                         skip_group_check=True)


def _fused(nc, tc, x, out, wqk, wvt, bqk, bv_t, identb, identv, ident,
           qTf, kTf, vT, v1, v18):
    """Fused setup+attention: x chunk j feeds kT/vT/qT chunk j; attention
    pair (g, p) needs only qT chunk g and kT chunk p//2, so two groups run
    concurrently with the x-load DMA and projections, hiding the ~38us
    HBM load under attention compute."""
    IDENT = mybir.ActivationFunctionType.Identity
    exp8 = _register_exp8()
    dr32 = _dr32()
    dve32 = int(os.environ.get("KERNEL_DVE32", "15"))
    look = int(os.environ.get("KERNEL_LOOK", "2"))
    NCH = NG  # 8 chunks of 512 t-rows

    with (
        tc.tile_pool(name="xin", bufs=3) as xin,
        tc.tile_pool(name="xT_pool", bufs=1) as xT_pool,
        tc.tile_pool(name="ps_mix", bufs=2, space="PSUM") as ps_mix,
        tc.tile_pool(name="ps_st", bufs=2, space="PSUM") as ps_st,
        tc.tile_pool(name="ps_out", bufs=2, space="PSUM") as ps_out,
        tc.tile_pool(name="expp", bufs=6) as expp,
        tc.tile_pool(name="outsb", bufs=2) as outsb,
        tc.tile_pool(name="stage", bufs=2) as stage,
        tc.tile_pool(name="recp", bufs=4) as recp,
    ):
        xT = xT_pool.tile([P, NE, T], BF16, tag="xT")
        if v1 is not None:
            nc.gpsimd.memset(v1[:, :, :, H:H + 1], 1.0)
        if v18 is not None:
            nc.gpsimd.memset(v18[:, :, :, H:H + 1], 1.0)
            nc.gpsimd.memset(v18[:, :, :, H + 1:VW8], 0.0)

        state = {"idx": 0}
        outps = {}
        ptr = {g: 0 for g in range(NG)}
        inflight = []
        admit_next = [0]
        pending = []

        def admit(j):
            while admit_next[0] <= min(j, NG - 1) and len(inflight) < 2:
                g = admit_next[0]
                inflight.append(g)
                ow = VW8 if dr32 > 0 else H + 1
                outps[g] = ps_out.tile([ow, GQ], F32, tag="o",
                                       name=f"outp{g}")
                admit_next[0] += 1

        def emit_mm1_exp(g, p):
            idx = state["idx"]
            state["idx"] += 1
            gsl = slice(g * GQ, (g + 1) * GQ)
            s0, s1 = 2 * p, 2 * p + 1
            stp = ps_st.tile([P, 2, 512], F32, tag="st", name=f"st{idx}")
            nc.tensor.matmul(stp[:, 0, :], kTf[0:H, s0 * P:(s0 + 1) * P],
                             qTf[0:H, gsl], start=True, stop=True)
            nc.tensor.matmul(stp[:, 1, :], qTf[H:P, s1 * P:(s1 + 1) * P],
                             kTf[H:P, gsl], start=True, stop=True)
            pf8 = _pair_fp8(idx, dr32)
            ex = expp.tile([P, 2, 512], FP8 if pf8 else BF16,
                           tag="ex8" if pf8 else "exb")
            _emit_exp(nc, exp8, ex, stp, idx, dve32)
            pending.append((g, p, ex, pf8))

        def flush_mm2(limit):
            while len(pending) > limit:
                g, p, ex, pf8 = pending.pop(0)
                _mm2(nc, outps[g], v1, v18, ex, p, pf8)

        def drain(g):
            osb = outsb.tile([H + 1, GQ], F32, tag="osb", name=f"osb{g}")
            nc.scalar.copy(osb, outps.pop(g)[0:H + 1, :])
            _attn_tail(nc, out, ident, osb, stage, recp, ps_mix, g,
                       tag="mix", width=512)
            inflight.remove(g)

        def pump(avail):
            progressed = True
            while progressed:
                progressed = False
                for g in list(inflight):
                    if ptr[g] < avail:
                        emit_mm1_exp(g, ptr[g])
                        ptr[g] += 1
                        flush_mm2(look)
                        progressed = True

        for j in range(NCH):
            admit(j - 1)
            if j >= 1:
                pump(2 * (j - 1) + 2 if (j - 1) % 2 == 1 else 2 * (j - 1))
            jsl = slice(j * 512, (j + 1) * 512)
            xt = xin.tile([P, 4, E], BF16, tag="x")
            if j == 0:
                # split the first load so the transpose pipeline starts as
                # soon as the first 128 rows land
                nc.gpsimd.dma_start(
                    out=xt[:, 0:1, :],
                    in_=x[0:P, :].rearrange("(i p) e -> p i e", p=P))
                nc.gpsimd.dma_start(
                    out=xt[:, 1:4, :],
                    in_=x[P:512, :].rearrange("(i p) e -> p i e", p=P))
            else:
                nc.gpsimd.dma_start(
                    out=xt,
                    in_=x[j * 512:(j + 1) * 512, :].rearrange(
                        "(i p) e -> p i e", p=P),
                )
            for i_sub in range(4):
                i = 4 * j + i_sub
                pmix = ps_mix.tile([P, 512], F32, tag="mix")
                pb = pmix.bitcast(BF16)
                for c in range(NE):
                    nc.tensor.transpose(
                        pb[:, c * P:(c + 1) * P],
                        xt[:, i_sub, c * P:(c + 1) * P], identb)
                nc.vector.tensor_copy(
                    xT[:, :, i * P:(i + 1) * P],
                    pb[:, 0:NE * P].rearrange("p (c q) -> p c q", q=P))

            pq = ps_mix.tile([P, 512], F32, tag="mix")
            for c in range(NE):
                nc.tensor.matmul(pq, wqk[:, c, :], xT[:, c, jsl],
                                 start=(c == 0), stop=(c == NE - 1))
            # qTf holds [q_lo; k_hi]; kTf holds the mirrors [k_lo; q_hi]
            nc.scalar.activation(out=qTf[:, jsl], in_=pq,
                                 func=IDENT, bias=bqk, scale=1.0)
            nc.sync.dma_start(out=kTf[H:P, jsl], in_=qTf[0:H, jsl])
            nc.sync.dma_start(out=kTf[0:H, jsl], in_=qTf[H:P, jsl])

            if j % 2 == 1:
                # col-tiled v projection for chunks j-1 (cols 0:64, psum
                # partitions 0:64) and j (cols 64:128, partitions 64:128) -
                # the two 64-wide matmuls stream concurrently
                jsl0 = slice((j - 1) * 512, j * 512)
                pv = ps_mix.tile([P, 512], F32, tag="mix")
                for c in range(NE):
                    nc.tensor.matmul(pv[0:H, :], wvt[:, c, 0:H],
                                     xT[:, c, jsl0],
                                     start=(c == 0), stop=(c == NE - 1),
                                     skip_group_check=True)
                    nc.tensor.matmul(pv[H:P, :], wvt[:, c, H:P],
                                     xT[:, c, jsl],
                                     start=(c == 0), stop=(c == NE - 1),
                                     skip_group_check=True)
                nc.scalar.activation(out=vT[0:H, jsl0], in_=pv[0:H, :],
                                     func=IDENT, bias=bv_t[0:H, :], scale=1.0)
                nc.scalar.activation(out=vT[H:P, jsl], in_=pv[H:P, :],
                                     func=IDENT, bias=bv_t[H:P, :], scale=1.0)
                # mirror chunk j's v rows down to partitions 0:64 so all
                # v1 transposes use the standard full-array path
                nc.sync.dma_start(out=vT[0:H, jsl], in_=vT[H:P, jsl])
                pmv = ps_mix.tile([P, 512], F32, tag="mix")
                pvb = pmv.bitcast(BF16)
                for si in range(8):
                    s0 = 4 * (j - 1) + si
                    nc.tensor.transpose(
                        pvb[:, si * H:(si + 1) * H],
                        vT[0:H, s0 * P:(s0 + 1) * P], identv[0:H, 0:H])
                vsl = slice(2 * (j - 1), 2 * (j - 1) + 4)
                pvr = pvb[:, 0:8 * H].rearrange("p (a b c) -> p a b c",
                                                b=2, c=H)
                if v1 is not None:
                    nc.vector.tensor_copy(v1[:, vsl, :, 0:H], pvr)
                if v18 is not None:
                    nc.vector.tensor_copy(v18[:, vsl, :, 0:H], pvr)

            admit(j)
            if j < NCH - 1:
                avail = 2 * j + 2 if j % 2 == 1 else 2 * j
                progressed = True
                while progressed:
                    progressed = False
                    for g in list(inflight):
                        if ptr[g] < avail:
                            emit_mm1_exp(g, ptr[g])
                            ptr[g] += 1
                            flush_mm2(look)
                            progressed = True
            else:
                while inflight or admit_next[0] < NG:
                    admit(NCH - 1)
                    progressed = False
                    for g in list(inflight):
                        if ptr[g] < NPAIR:
                            emit_mm1_exp(g, ptr[g])
                            ptr[g] += 1
                            flush_mm2(look)
                            progressed = True
                        elif all(pg != g for pg, _, _, _ in pending):
                            drain(g)
                            progressed = True
                    if not progressed:
                        flush_mm2(0)


def _setup(nc, tc, x, wqk, wvt, bqk, bv_t, identb, identv, qTf, kTf, vT,
           v1, v18):
    IDENT = mybir.ActivationFunctionType.Identity
    # ---------------- phase 1: load x (bf16 cast) and transpose to xT ------
    with (
        tc.tile_pool(name="xT_pool", bufs=1) as xT_pool,
        tc.tile_pool(name="xin", bufs=3) as xin,
        tc.tile_pool(name="ps_t", bufs=4, space="PSUM") as ps_t,
        tc.tile_pool(name="ps_p", bufs=2, space="PSUM") as ps_p,
    ):
        xT = xT_pool.tile([P, NE, T], BF16, tag="xT")
        SUB = int(os.environ.get("KERNEL_CAST_SUB", "8"))
        for k in range(NT // SUB):
            xt = xin.tile([P, SUB, E], BF16, tag="x")
            nc.gpsimd.dma_start(
                out=xt,
                in_=x[k * SUB * P:(k + 1) * SUB * P, :].rearrange(
                    "(i p) e -> p i e", p=P),
            )
            for i_sub in range(SUB):
                i = k * SUB + i_sub
                for c4 in range(NE // 4 + (1 if NE % 4 else 0)):
                    cs = list(range(c4 * 4, min(NE, (c4 + 1) * 4)))
                    pst = ps_t.tile([P, 4, P], BF16, tag="t")
                    for ci, c in enumerate(cs):
                        nc.tensor.transpose(
                            pst[:, ci, :],
                            xt[:, i_sub, c * P:(c + 1) * P], identb)
                    nc.vector.tensor_copy(
                        xT[:, cs[0]:cs[-1] + 1, i * P:(i + 1) * P],
                        pst[:, 0:len(cs), :])

        # ---------------- phase 2: projections -> qT/kT/vT + mirrors -------
        for j in range(T // 512):
            jsl = slice(j * 512, (j + 1) * 512)
            psqk = ps_p.tile([P, 512], F32, tag="pqk")
            for c in range(NE):
                nc.tensor.matmul(
                    psqk, wqk[:, c, :], xT[:, c, jsl],
                    start=(c == 0), stop=(c == NE - 1),
                )
            # qTf holds [q_lo; k_hi]; kTf gets the mirrors [k_lo; q_hi]
            # (same layout as _fused - mm1 relies on it)
            nc.scalar.activation(
                out=qTf[:, jsl], in_=psqk, func=IDENT, bias=bqk, scale=1.0,
            )
            psv = ps_p.tile([H, 512], F32, tag="pv")
            for c in range(NE):
                nc.tensor.matmul(
                    psv, wvt[:, c, 0:H], xT[:, c, jsl],
                    start=(c == 0), stop=(c == NE - 1),
                )
            nc.scalar.activation(
                out=vT[0:H, jsl], in_=psv, func=IDENT, bias=bv_t[0:H, :],
                scale=1.0,
            )
            # mirror q to partitions 64:128 and k to 0:64 for row-packed MM1
            nc.sync.dma_start(out=kTf[H:P, jsl], in_=qTf[0:H, jsl])
            nc.sync.dma_start(out=kTf[0:H, jsl], in_=qTf[H:P, jsl])

    # ---------------- phase 3: vT -> v1 tiles [128, 2, 65] -----------------
    with tc.tile_pool(name="ps_v", bufs=2, space="PSUM") as ps_v:
        if v1 is not None:
            nc.gpsimd.memset(v1[:, :, :, H:H + 1], 1.0)
        if v18 is not None:
            nc.gpsimd.memset(v18[:, :, :, H:H + 1], 1.0)
            nc.gpsimd.memset(v18[:, :, :, H + 1:VW8], 0.0)
        idv = identv[0:H, 0:H]
        for s in range(NT):
            psv = ps_v.tile([P, H], BF16, tag="v")
            nc.tensor.transpose(psv, vT[0:H, s * P:(s + 1) * P], idv)
            if v1 is not None:
                nc.vector.tensor_copy(v1[:, s // 2, s % 2, 0:H], psv)
            if v18 is not None:
                nc.vector.tensor_copy(v18[:, s // 2, s % 2, 0:H], psv)


def _attention(nc, tc, out, ident, qTf, kTf, v1, v18):
    exp8 = _register_exp8()
    dr32 = _dr32()
    dve32 = int(os.environ.get("KERNEL_DVE32", "15"))
    lookahead = int(os.environ.get("KERNEL_LOOKAHEAD", "2"))
    with (
        tc.tile_pool(name="ps_st", bufs=2, space="PSUM") as ps_st,
        tc.tile_pool(name="ps_out", bufs=2, space="PSUM") as ps_out,
        tc.tile_pool(name="expp", bufs=6) as expp,
        tc.tile_pool(name="outsb", bufs=2) as outsb,
        tc.tile_pool(name="stage", bufs=2) as stage,
        tc.tile_pool(name="ps_tail", bufs=2, space="PSUM") as ps_tail,
        tc.tile_pool(name="recp", bufs=4) as recp,
    ):
        def mm1(g, p):
            gsl = slice(g * GQ, (g + 1) * GQ)
            s0, s1 = 2 * p, 2 * p + 1
            stp = ps_st.tile([P, 2, 512], F32, tag="st")
            nc.tensor.matmul(
                stp[:, 0, :], kTf[0:H, s0 * P:(s0 + 1) * P], qTf[0:H, gsl],
                start=True, stop=True,
            )
            nc.tensor.matmul(
                stp[:, 1, :], qTf[H:P, s1 * P:(s1 + 1) * P], kTf[H:P, gsl],
                start=True, stop=True,
            )
            return stp

        outps = {}
        it = [(g, p) for g in range(NG) for p in range(NPAIR)]
        stps = [mm1(*it[i]) for i in range(lookahead)]
        for idx, (g, p) in enumerate(it):
            if p == 0:
                ow = VW8 if dr32 > 0 else H + 1
                outps[g] = ps_out.tile([ow, GQ], F32, tag="o",
                                       name=f"outp{g}")
            stp = stps.pop(0) if lookahead else mm1(g, p)
            pf8 = _pair_fp8(idx, dr32)
            ex = expp.tile([P, 2, 512], FP8 if pf8 else BF16,
                           tag="ex8" if pf8 else "exb")
            _emit_exp(nc, exp8, ex, stp, idx, dve32)
            if lookahead and idx + lookahead < len(it):
                stps.append(mm1(*it[idx + lookahead]))
            _mm2(nc, outps[g], v1, v18, ex, p, pf8)
            if p == NPAIR - 1:
                osb = outsb.tile([H + 1, GQ], F32, tag="osb", name=f"osb{g}")
                nc.scalar.copy(osb, outps.pop(g)[0:H + 1, :])
                _attn_tail(nc, out, ident, osb, stage, recp, ps_tail, g)


def _attn_tail(nc, out, ident, osb, stage, recp, ps_pool, g,
               tag="st", width=H + 1):
    nb = GQ // P
    st_t = stage.tile([P, nb, H], F32, tag="stage", name=f"st_t{g}")
    for b in range(nb):
        pst_t = ps_pool.tile([P, width], F32, tag=tag, name=f"pst{g}_{b}")
        pst = pst_t[:, 0:H + 1]
        nc.tensor.transpose(
            pst, osb[:, b * P:(b + 1) * P], ident[0:H + 1, 0:H + 1]
        )
        rec = recp.tile([P, 1], F32, tag="rec")
        nc.vector.reciprocal(rec, pst[:, H:H + 1])
        nc.vector.tensor_scalar_mul(st_t[:, b, :], pst[:, 0:H], rec)
    nc.sync.dma_start(
        out=out[g * GQ:(g + 1) * GQ, :].rearrange("(b p) h -> p b h", p=P),
        in_=st_t,
    )


_NC_CACHE = {}


def _get_nc():
    if "nc" not in _NC_CACHE:
        _NC_CACHE["nc"] = build_nc()
    return _NC_CACHE["nc"]


def kernel(x, Wq, bq, Wk, bk, Wv, bv):
    x = np.ascontiguousarray(np.asarray(x, dtype=np.float32))
    in_common = {
        "Wq": np.ascontiguousarray(np.asarray(Wq, np.float32)),
        "Wk": np.ascontiguousarray(np.asarray(Wk, np.float32)),
        "Wv": np.ascontiguousarray(np.asarray(Wv, np.float32)),
        "bq": np.ascontiguousarray(np.asarray(bq, np.float32)),
        "bk": np.ascontiguousarray(np.asarray(bk, np.float32)),
        "bv": np.ascontiguousarray(np.asarray(bv, np.float32)),
    }
    nc = _get_nc()
    in_maps = [dict(in_common, x=x[b]) for b in range(B)]
    res = run_bass_kernel_spmd(nc, in_maps, core_ids=list(range(B)))
    return np.stack([res.results[b]["out"] for b in range(B)], axis=0)


if __name__ == "__main__":
    rng = np.random.default_rng(0)
    xs = rng.standard_normal((B, T, E), dtype=np.float32)
    s = 1.0 / np.sqrt(E)
    mk = lambda *shape: rng.uniform(-s, s, size=shape).astype(np.float32)
    o = kernel(xs, mk(E, H), mk(H), mk(E, H), mk(H), mk(E, H), mk(H))
    print("out", o.shape, o.dtype, float(np.abs(o).max()))
